# revision 1
# baseline (speedup 1.0000x reference)
"""Expert-parallel MoE MLP kernel for TRN2 (8 NeuronCores).

Reference computation (all experts, dense routing):
    hidden = einsum("bnd,edh->benh", x, w1); hidden = gelu(hidden)
    out    = einsum("benh,ehd->bnde", hidden, w2)        # [b, n, d4, e]

Sharding: expert-parallel, 2 experts per core (16 experts / 8 cores); x is
replicated. Each core computes, for its experts e:
    hT[e] = gelu(W1[e].T @ X.T)        # [h, tok] layout, h on partitions
    outT[e] = W2[e].T @ hT[e]          # [d4, tok] layout
which keeps the contraction dim on SBUF partitions for both matmuls with no
on-device transposes: W1 (d,h) / W2 (h,d4) load in natural layout as lhsT, and
X.T is prepared once on the host. Matmuls run as float32r (single-pass reduced
precision fp32, ~1.5e-4 rel err) — 4x the throughput of strict fp32 on the PE.
The [e, d4, tok] device layout is re-interleaved to [b, n, d4, e] on the host.
"""

import sys

import numpy as np

for _p in ("/opt/trn_rl_repo", "/root/.axon_site/_ro/trn_rl_repo"):
    if _p not in sys.path:
        sys.path.append(_p)

import concourse.bacc as bacc
import concourse.mybir as mybir
import concourse.tile as tile
from concourse.bass_utils import run_bass_kernel_spmd

F32 = mybir.dt.float32
F32R = mybir.dt.float32r

N_CORES = 8
E = 16                 # total experts
E_LOC = E // N_CORES   # experts per core
D = 512                # model dim (contraction of mm1)
H = 512                # hidden dim (contraction of mm2)
D4 = 128               # output dim per expert
NTOK = 4 * 2048        # tokens
TT = 512               # token tile (matmul moving free dim)
P = 128


def _build_program():
    nc = bacc.Bacc("TRN2", target_bir_lowering=False, debug=False)
    xT = nc.declare_dram_parameter("xT", [D, NTOK], F32R, isOutput=False)
    w1 = nc.declare_dram_parameter("w1", [E_LOC, D, H], F32R, isOutput=False)
    w2 = nc.declare_dram_parameter("w2", [E_LOC, H, D4], F32R, isOutput=False)
    outT = nc.declare_dram_parameter("outT", [E_LOC, D4, NTOK], F32, isOutput=True)

    gelu = mybir.ActivationFunctionType.Gelu
    n_dt = D // P   # 4 k-tiles of mm1
    n_ht = H // P   # 4 k-tiles of mm2

    with tile.TileContext(nc) as tc:
        with (
            tc.tile_pool(name="wpool", bufs=1) as wpool,
            tc.tile_pool(name="xpool", bufs=3) as xpool,
            tc.tile_pool(name="hpool", bufs=2) as hpool,
            tc.tile_pool(name="opool", bufs=3) as opool,
            tc.tile_pool(name="ps1p", bufs=2, space="PSUM") as ps1p,
            tc.tile_pool(name="ps2p", bufs=2, space="PSUM") as ps2p,
        ):
            # Weights resident in SBUF for the whole kernel, natural layout.
            w1_sb = wpool.tile([P, E_LOC, n_dt, H], F32R, name="w1_sb", tag="w1")
            nc.sync.dma_start(w1_sb, w1.rearrange("e (dt p) h -> p e dt h", p=P))
            w2_sb = wpool.tile([P, E_LOC, n_ht, D4], F32R, name="w2_sb", tag="w2")
            nc.sync.dma_start(w2_sb, w2.rearrange("e (ht p) d -> p e ht d", p=P))

            xT_r = xT.rearrange("(dt p) n -> p dt n", p=P)

            for t in range(NTOK // TT):
                tok = slice(t * TT, (t + 1) * TT)
                x_sb = xpool.tile([P, n_dt, TT], F32R, name="x_sb", tag="x")
                nc.sync.dma_start(x_sb, xT_r[:, :, tok])
                for e in range(E_LOC):
                    hT_sb = hpool.tile([P, n_ht, TT], F32R, name="hT_sb", tag="h")
                    for ht in range(n_ht):
                        ps1 = ps1p.tile([P, TT], F32, name="ps1", tag="ps1")
                        for dt_i in range(n_dt):
                            nc.tensor.matmul(
                                ps1,
                                w1_sb[:, e, dt_i, ht * P : (ht + 1) * P],
                                x_sb[:, dt_i, :],
                                start=(dt_i == 0),
                                stop=(dt_i == n_dt - 1),
                            )
                        nc.scalar.activation(hT_sb[:, ht, :], ps1, gelu)
                    ps2 = ps2p.tile([P, TT], F32, name="ps2", tag="ps2")
                    for ht in range(n_ht):
                        nc.tensor.matmul(
                            ps2,
                            w2_sb[:, e, ht, :],
                            hT_sb[:, ht, :],
                            start=(ht == 0),
                            stop=(ht == n_ht - 1),
                        )
                    o_sb = opool.tile([P, TT], F32, name="o_sb", tag="o")
                    nc.vector.tensor_copy(o_sb, ps2)
                    nc.sync.dma_start(outT[e, :, tok], o_sb)

    nc.finalize()
    return nc


_NC = None


def _get_program():
    global _NC
    if _NC is None:
        _NC = _build_program()
    return _NC


def kernel(x: np.ndarray, w1: np.ndarray, w2: np.ndarray, **_) -> np.ndarray:
    """Full inputs in, full output out; expert-parallel across 8 NeuronCores."""
    nc = _get_program()

    xT = np.ascontiguousarray(x.reshape(NTOK, D).T).astype(np.float32, copy=False)
    in_maps = [
        {
            "xT": xT,
            "w1": np.ascontiguousarray(w1[c * E_LOC : (c + 1) * E_LOC]).astype(
                np.float32, copy=False
            ),
            "w2": np.ascontiguousarray(w2[c * E_LOC : (c + 1) * E_LOC]).astype(
                np.float32, copy=False
            ),
        }
        for c in range(N_CORES)
    ]
    res = run_bass_kernel_spmd(nc, in_maps, list(range(N_CORES)))

    full = np.stack([res.results[c]["outT"] for c in range(N_CORES)], axis=0)
    full = full.reshape(E, D4, NTOK)              # [e, d4, tok]
    out = full.transpose(2, 1, 0)                 # [tok, d4, e]
    return np.ascontiguousarray(out.reshape(4, 2048, D4, E), dtype=np.float32)


# revision 2
# speedup vs baseline: 1.0552x; 1.0552x over previous
"""Expert-parallel MoE MLP kernel for TRN2 (8 NeuronCores).

Reference computation (all experts, dense routing):
    hidden = einsum("bnd,edh->benh", x, w1); hidden = gelu(hidden)
    out    = einsum("benh,ehd->bnde", hidden, w2)        # [b, n, d4, e]

Sharding: expert-parallel, 2 experts per core (16 experts / 8 cores); x is
replicated. Each core computes, for its experts e:
    hT[e] = gelu(W1[e].T @ X.T)        # [h, tok] layout, h on partitions
    outT[e] = W2[e].T @ hT[e]          # [d4, tok] layout
which keeps the contraction dim on SBUF partitions for both matmuls with no
on-device transposes: W1 (d,h) / W2 (h,d4) load in natural layout as lhsT, and
X.T is prepared once on the host. Matmuls run as float32r (single-pass reduced
precision fp32, ~1.5e-4 rel err) — 4x the throughput of strict fp32 on the PE.
The [e, d4, tok] device layout is re-interleaved to [b, n, d4, e] on the host.
"""

import sys

import numpy as np

for _p in ("/opt/trn_rl_repo", "/root/.axon_site/_ro/trn_rl_repo"):
    if _p not in sys.path:
        sys.path.append(_p)

import concourse.bacc as bacc
import concourse.mybir as mybir
import concourse.tile as tile
from concourse.bass_utils import run_bass_kernel_spmd

F32 = mybir.dt.float32
F32R = mybir.dt.float32r

N_CORES = 8
E = 16                 # total experts
E_LOC = E // N_CORES   # experts per core
D = 512                # model dim (contraction of mm1)
H = 512                # hidden dim (contraction of mm2)
D4 = 128               # output dim per expert
NTOK = 4 * 2048        # tokens
TT = 512               # token tile (matmul moving free dim)
P = 128


def _build_program():
    nc = bacc.Bacc("TRN2", target_bir_lowering=False, debug=False)
    xT = nc.declare_dram_parameter("xT", [D, NTOK], F32R, isOutput=False)
    w1 = nc.declare_dram_parameter("w1", [E_LOC, D, H], F32R, isOutput=False)
    w2 = nc.declare_dram_parameter("w2", [E_LOC, H, D4], F32R, isOutput=False)
    outT = nc.declare_dram_parameter("outT", [E_LOC, D4, NTOK], F32, isOutput=True)

    gelu = mybir.ActivationFunctionType.Gelu
    n_dt = D // P   # 4 k-tiles of mm1
    n_ht = H // P   # 4 k-tiles of mm2

    with tile.TileContext(nc) as tc:
        with (
            tc.tile_pool(name="wpool", bufs=1) as wpool,
            tc.tile_pool(name="xpool", bufs=3) as xpool,
            tc.tile_pool(name="hpool", bufs=2) as hpool,
            tc.tile_pool(name="opool", bufs=3) as opool,
            tc.tile_pool(name="ps1p", bufs=4, space="PSUM") as ps1p,
            tc.tile_pool(name="ps2p", bufs=2, space="PSUM") as ps2p,
        ):
            # Weights resident in SBUF for the whole kernel, natural layout.
            # Loaded as one DMA per (e, k-tile) so the first matmuls only wait
            # on the slices they read instead of the whole weight block.
            w1_sb = wpool.tile([P, E_LOC, n_dt, H], F32R, name="w1_sb", tag="w1")
            w1_r = w1.rearrange("e (dt p) h -> p e dt h", p=P)
            for e in range(E_LOC):
                for dt_i in range(n_dt):
                    nc.sync.dma_start(w1_sb[:, e, dt_i], w1_r[:, e, dt_i])
            w2_sb = wpool.tile([P, E_LOC, n_ht, D4], F32R, name="w2_sb", tag="w2")
            w2_r = w2.rearrange("e (ht p) d -> p e ht d", p=P)
            for e in range(E_LOC):
                nc.sync.dma_start(w2_sb[:, e], w2_r[:, e])

            xT_r = xT.rearrange("(dt p) n -> p dt n", p=P)

            for t in range(NTOK // TT):
                tok = slice(t * TT, (t + 1) * TT)
                # one tile per d k-tile so mm1 starts as soon as its slice lands
                x_sb = [
                    xpool.tile([P, TT], F32R, name=f"x{dt_i}_sb", tag=f"x{dt_i}")
                    for dt_i in range(n_dt)
                ]
                for dt_i in range(n_dt):
                    nc.sync.dma_start(x_sb[dt_i], xT_r[:, dt_i, tok])
                for e in range(E_LOC):
                    hT_sb = hpool.tile([P, n_ht, TT], F32R, name="hT_sb", tag="h")
                    for ht in range(n_ht):
                        ps1 = ps1p.tile([P, TT], F32, name="ps1", tag="ps1")
                        for dt_i in range(n_dt):
                            nc.tensor.matmul(
                                ps1,
                                w1_sb[:, e, dt_i, ht * P : (ht + 1) * P],
                                x_sb[dt_i],
                                start=(dt_i == 0),
                                stop=(dt_i == n_dt - 1),
                            )
                        nc.scalar.activation(hT_sb[:, ht, :], ps1, gelu)
                    ps2 = ps2p.tile([P, TT], F32, name="ps2", tag="ps2")
                    for ht in range(n_ht):
                        nc.tensor.matmul(
                            ps2,
                            w2_sb[:, e, ht, :],
                            hT_sb[:, ht, :],
                            start=(ht == 0),
                            stop=(ht == n_ht - 1),
                        )
                    o_sb = opool.tile([P, TT], F32, name="o_sb", tag="o")
                    nc.vector.tensor_copy(o_sb, ps2)
                    nc.sync.dma_start(outT[e, :, tok], o_sb)

    nc.finalize()
    return nc


_NC = None


def _get_program():
    global _NC
    if _NC is None:
        _NC = _build_program()
    return _NC


def kernel(x: np.ndarray, w1: np.ndarray, w2: np.ndarray, **_) -> np.ndarray:
    """Full inputs in, full output out; expert-parallel across 8 NeuronCores."""
    nc = _get_program()

    xT = np.ascontiguousarray(x.reshape(NTOK, D).T).astype(np.float32, copy=False)
    in_maps = [
        {
            "xT": xT,
            "w1": np.ascontiguousarray(w1[c * E_LOC : (c + 1) * E_LOC]).astype(
                np.float32, copy=False
            ),
            "w2": np.ascontiguousarray(w2[c * E_LOC : (c + 1) * E_LOC]).astype(
                np.float32, copy=False
            ),
        }
        for c in range(N_CORES)
    ]
    res = run_bass_kernel_spmd(nc, in_maps, list(range(N_CORES)))

    full = np.stack([res.results[c]["outT"] for c in range(N_CORES)], axis=0)
    full = full.reshape(E, D4, NTOK)              # [e, d4, tok]
    out = full.transpose(2, 1, 0)                 # [tok, d4, e]
    return np.ascontiguousarray(out.reshape(4, 2048, D4, E), dtype=np.float32)


# revision 3
# speedup vs baseline: 1.1176x; 1.0591x over previous
"""Expert-parallel MoE MLP kernel for TRN2 (8 NeuronCores).

Reference computation (all experts, dense routing):
    hidden = einsum("bnd,edh->benh", x, w1); hidden = gelu(hidden)
    out    = einsum("benh,ehd->bnde", hidden, w2)        # [b, n, d4, e]

Sharding: expert-parallel, 2 experts per core (16 experts / 8 cores); x is
replicated. Each core computes, for its experts e:
    hT[e] = gelu(W1[e].T @ X.T)        # [h, tok] layout, h on partitions
    outT[e] = W2[e].T @ hT[e]          # [d4, tok] layout
which keeps the contraction dim on SBUF partitions for both matmuls with no
on-device transposes: W1 (d,h) / W2 (h,d4) load in natural layout as lhsT, and
X.T is prepared once on the host. Matmuls run as float32r (single-pass reduced
precision fp32, ~1.5e-4 rel err) — 4x the throughput of strict fp32 on the PE.
The [e, d4, tok] device layout is re-interleaved to [b, n, d4, e] on the host.
"""

import sys

import numpy as np

for _p in ("/opt/trn_rl_repo", "/root/.axon_site/_ro/trn_rl_repo"):
    if _p not in sys.path:
        sys.path.append(_p)

import concourse.bacc as bacc
import concourse.mybir as mybir
import concourse.tile as tile
from concourse.bass_utils import run_bass_kernel_spmd

F32 = mybir.dt.float32
F32R = mybir.dt.float32r

N_CORES = 8
E = 16                 # total experts
E_LOC = E // N_CORES   # experts per core
D = 512                # model dim (contraction of mm1)
H = 512                # hidden dim (contraction of mm2)
D4 = 128               # output dim per expert
NTOK = 4 * 2048        # tokens
TT = 512               # token tile (matmul moving free dim)
P = 128


def _build_program():
    nc = bacc.Bacc("TRN2", target_bir_lowering=False, debug=False)
    xT = nc.declare_dram_parameter("xT", [D, NTOK], F32R, isOutput=False)
    w1 = nc.declare_dram_parameter("w1", [E_LOC, D, H], F32R, isOutput=False)
    w2 = nc.declare_dram_parameter("w2", [E_LOC, H, D4], F32R, isOutput=False)
    outT = nc.declare_dram_parameter("outT", [E_LOC, D4, NTOK], F32, isOutput=True)

    gelu = mybir.ActivationFunctionType.Gelu
    n_dt = D // P   # 4 k-tiles of mm1
    n_ht = H // P   # 4 k-tiles of mm2

    with tile.TileContext(nc) as tc:
        with (
            tc.tile_pool(name="wpool", bufs=1) as wpool,
            tc.tile_pool(name="xpool", bufs=3) as xpool,
            tc.tile_pool(name="hpool", bufs=2) as hpool,
            tc.tile_pool(name="opool", bufs=3) as opool,
            tc.tile_pool(name="ps1p", bufs=4, space="PSUM") as ps1p,
            tc.tile_pool(name="ps2p", bufs=2, space="PSUM") as ps2p,
        ):
            # Weights resident in SBUF for the whole kernel, natural layout.
            # DMA issue order puts the first token tile's x slices ahead of the
            # weights (and e0's weights ahead of e1's) so the first matmul only
            # waits on ~0.5MB instead of the whole 3.6MB input block.
            w1_sb = wpool.tile([P, E_LOC, n_dt, H], F32R, name="w1_sb", tag="w1")
            w1_r = w1.rearrange("e (dt p) h -> p e dt h", p=P)
            w2_sb = wpool.tile([P, E_LOC, n_ht, D4], F32R, name="w2_sb", tag="w2")
            w2_r = w2.rearrange("e (ht p) d -> p e ht d", p=P)
            xT_r = xT.rearrange("(dt p) n -> p dt n", p=P)

            x_tiles = {}

            def load_x(t):
                tok = slice(t * TT, (t + 1) * TT)
                x_sb = [
                    xpool.tile([P, TT], F32R, name=f"x{dt_i}_sb", tag=f"x{dt_i}")
                    for dt_i in range(n_dt)
                ]
                for dt_i in range(n_dt):
                    nc.sync.dma_start(x_sb[dt_i], xT_r[:, dt_i, tok])
                x_tiles[t] = x_sb

            load_x(0)
            for e in range(E_LOC):
                for dt_i in range(n_dt):
                    nc.sync.dma_start(w1_sb[:, e, dt_i], w1_r[:, e, dt_i])
                nc.sync.dma_start(w2_sb[:, e], w2_r[:, e])

            for t in range(NTOK // TT):
                tok = slice(t * TT, (t + 1) * TT)
                if t not in x_tiles:
                    load_x(t)
                x_sb = x_tiles.pop(t)
                hT_tiles = []
                for e in range(E_LOC):
                    hT_sb = hpool.tile([P, n_ht, TT], F32R, name="hT_sb", tag="h")
                    for ht in range(n_ht):
                        ps1 = ps1p.tile([P, TT], F32, name="ps1", tag="ps1")
                        for dt_i in range(n_dt):
                            nc.tensor.matmul(
                                ps1,
                                w1_sb[:, e, dt_i, ht * P : (ht + 1) * P],
                                x_sb[dt_i],
                                start=(dt_i == 0),
                                stop=(dt_i == n_dt - 1),
                            )
                        nc.scalar.activation(hT_sb[:, ht, :], ps1, gelu)
                    hT_tiles.append(hT_sb)
                for e in range(E_LOC):
                    ps2 = ps2p.tile([P, TT], F32, name="ps2", tag="ps2")
                    for ht in range(n_ht):
                        nc.tensor.matmul(
                            ps2,
                            w2_sb[:, e, ht, :],
                            hT_tiles[e][:, ht, :],
                            start=(ht == 0),
                            stop=(ht == n_ht - 1),
                        )
                    o_sb = opool.tile([P, TT], F32, name="o_sb", tag="o")
                    nc.vector.tensor_copy(o_sb, ps2)
                    nc.sync.dma_start(outT[e, :, tok], o_sb)

    nc.finalize()
    return nc


_NC = None


def _get_program():
    global _NC
    if _NC is None:
        _NC = _build_program()
    return _NC


def kernel(x: np.ndarray, w1: np.ndarray, w2: np.ndarray, **_) -> np.ndarray:
    """Full inputs in, full output out; expert-parallel across 8 NeuronCores."""
    nc = _get_program()

    xT = np.ascontiguousarray(x.reshape(NTOK, D).T).astype(np.float32, copy=False)
    in_maps = [
        {
            "xT": xT,
            "w1": np.ascontiguousarray(w1[c * E_LOC : (c + 1) * E_LOC]).astype(
                np.float32, copy=False
            ),
            "w2": np.ascontiguousarray(w2[c * E_LOC : (c + 1) * E_LOC]).astype(
                np.float32, copy=False
            ),
        }
        for c in range(N_CORES)
    ]
    res = run_bass_kernel_spmd(nc, in_maps, list(range(N_CORES)))

    full = np.stack([res.results[c]["outT"] for c in range(N_CORES)], axis=0)
    full = full.reshape(E, D4, NTOK)              # [e, d4, tok]
    out = full.transpose(2, 1, 0)                 # [tok, d4, e]
    return np.ascontiguousarray(out.reshape(4, 2048, D4, E), dtype=np.float32)
